# revision 39
# baseline (speedup 1.0000x reference)
"""NTM scatter_memory kernel for Trainium2 (8 NeuronCores, data-parallel over batch).

Computation per batch b (B=256, N=2048, M=128, S=3):
  dot[n]   = sum_m k[m] * mem[n,m]
  cos[n]   = dot[n] / max(||k|| * ||mem[n]||, 1e-8)
  wc       = softmax(beta * cos)
  wg       = g*wc + (1-g)*prev_w
  wt[n]    = s0*wg[n-1] + s1*wg[n] + s2*wg[n+1]   (circular)
  w        = wt**y / sum(wt**y)
  read[m]  = sum_n w[n] mem[n,m]
  new_mem  = mem * (1 - w x e) + w x a

Per-core layout: 32 batches, processed in 8 groups of 4.  Memory tiles are
[128 partitions (n chunk), 4 batches x 128 (m)] so one n-chunk of 4 batches is
a single [128, 512] tile.  Engine split: GPSIMD does the dot elementwise
multiply, DVE the reductions + erase/add, ACT the norms (Square with
accum_out) and transcendentals, PE the outer products / read / shift /
transposes (float32r where free dim >= 256).
"""

import os
from contextlib import ExitStack

import numpy as np

import concourse.bacc as bacc
import concourse.bass as bass
import concourse.mybir as mybir
import concourse.tile as tile
from concourse.bass_utils import run_bass_kernel_spmd

B, N, M, S = 256, 2048, 128, 3
NCORES = 8
BC = B // NCORES          # batches per core
GB = 4                    # batches per group
NT = N // 128             # n chunks of 128
FD = GB * M               # 512, free dim of packed tiles
EPS = 1e-8

f32 = mybir.dt.float32
f32r = mybir.dt.float32r
AX = mybir.AxisListType
ALU = mybir.AluOpType
ACT = mybir.ActivationFunctionType

# scal tensor column indices
SC_KNORM, SC_BETA, SC_G, SC_OMG, SC_S0, SC_S1, SC_S2, SC_Y = range(8)
NSC = 8


def build_program(bc=BC):
    """Build the Bass program for one core processing `bc` batches."""
    ng = bc // GB
    nc = bacc.Bacc("TRN2", target_bir_lowering=False, debug=False)

    mem_d = nc.dram_tensor("memory", (bc, N, M), f32, kind="ExternalInput").ap()
    pwT_d = nc.dram_tensor("prevwT", (128, bc * NT), f32, kind="ExternalInput").ap()
    kbc_d = nc.dram_tensor("kbc", (128, bc * M), f32, kind="ExternalInput").ap()
    scal_d = nc.dram_tensor("scal", (128, bc * NSC), f32, kind="ExternalInput").ap()
    nege_d = nc.dram_tensor("negediag", (ng, GB, FD), f32, kind="ExternalInput").ap()
    adia_d = nc.dram_tensor("adiag", (ng, GB, FD), f32, kind="ExternalInput").ap()
    su_d = nc.dram_tensor("su", (128, 128), f32, kind="ExternalInput").ap()
    sd_d = nc.dram_tensor("sd", (128, 128), f32, kind="ExternalInput").ap()
    w127_d = nc.dram_tensor("w127", (128, 128), f32, kind="ExternalInput").ap()
    w0_d = nc.dram_tensor("w0", (128, 128), f32, kind="ExternalInput").ap()
    id_d = nc.dram_tensor("ident", (128, 128), f32, kind="ExternalInput").ap()
    ones_d = nc.dram_tensor("ones", (1, FD), f32, kind="ExternalInput").ap()
    onescol_d = nc.dram_tensor("onescol", (128, 1), f32, kind="ExternalInput").ap()

    newm_d = nc.dram_tensor("newmem", (bc, N, M), f32, kind="ExternalOutput").ap()
    wT_d = nc.dram_tensor("w_T", (128, bc * NT), f32, kind="ExternalOutput").ap()
    read_d = nc.dram_tensor("readout", (bc, M), f32, kind="ExternalOutput").ap()

    with tile.TileContext(nc) as tc, ExitStack() as ctx:
        cpool = ctx.enter_context(tc.tile_pool(name="consts", bufs=1))
        mpool = ctx.enter_context(tc.tile_pool(name="mem", bufs=3))
        prodp = ctx.enter_context(tc.tile_pool(name="prod", bufs=3))
        sqp = ctx.enter_context(tc.tile_pool(name="sq", bufs=2))
        smp = ctx.enter_context(tc.tile_pool(name="small", bufs=2))
        newp = ctx.enter_context(tc.tile_pool(name="new", bufs=3))
        edp = ctx.enter_context(tc.tile_pool(name="ediag", bufs=2))
        # PSUM pools: 8 banks total (nps 3 + rd 2 + wps 3)
        ppool = ctx.enter_context(tc.tile_pool(name="pps", bufs=3, space="PSUM"))
        rdpool = ctx.enter_context(tc.tile_pool(name="rdps", bufs=2, space="PSUM"))
        wpsp = ctx.enter_context(tc.tile_pool(name="wps", bufs=3, space="PSUM"))

        # persistent SBUF data
        su_sb = cpool.tile([128, 128], f32, tag="su")
        nc.sync.dma_start(su_sb[:], su_d)
        sd_sb = cpool.tile([128, 128], f32, tag="sd")
        nc.sync.dma_start(sd_sb[:], sd_d)
        w127_sb = cpool.tile([128, 128], f32, tag="w127")
        nc.sync.dma_start(w127_sb[:], w127_d)
        w0_sb = cpool.tile([128, 128], f32, tag="w0")
        nc.sync.dma_start(w0_sb[:], w0_d)
        id_sb = cpool.tile([128, 128], f32, tag="ident")
        nc.sync.dma_start(id_sb[:], id_d)
        ones_sb = cpool.tile([1, FD], f32, tag="ones")
        nc.sync.dma_start(ones_sb[:], ones_d)
        onescol_sb = cpool.tile([128, 1], f32, tag="onescol")
        nc.sync.dma_start(onescol_sb[:], onescol_d)

        pwT_sb = cpool.tile([128, bc * NT], f32, tag="pwT")
        nc.sync.dma_start(pwT_sb[:], pwT_d)
        scal_sb = cpool.tile([128, bc * NSC], f32, tag="scal")
        nc.sync.dma_start(scal_sb[:], scal_d)
        scal_v = scal_sb[:].rearrange("p (b j) -> p b j", j=NSC)

        ones_row = ones_sb[0:1, 0:128]
        # fp32r operands for the outer-product matmuls: walrus requires the
        # producer instruction to round to fp32r, so copy-convert once.
        ones_r = cpool.tile([1, FD], f32r, tag="ones_r")
        nc.scalar.copy(ones_r[:], ones_sb[:])

        for g in range(ng):
            b0 = g * GB

            def sc(j):
                # [128, GB] -> broadcast along t -> [128, GB, NT]
                return scal_v[:, b0:b0 + GB, j].broadcast_to((128, GB, NT))

            # ---- load group memory ----
            mem4 = mpool.tile([128, NT * FD], f32, tag="mem4")
            vm = mem4[:].rearrange("p (t b m) -> p t b m", t=NT, b=GB)
            with nc.named_scope("load"):
                for b in range(GB):
                    nc.sync.dma_start(
                        vm[:, :, b, :],
                        mem_d[b0 + b].rearrange("(t p) m -> p t m", p=128),
                    )
            kb_sb = edp.tile([128, FD], f32, tag="kb")
            nc.sync.dma_start(kb_sb[:], kbc_d[:, b0 * M:(b0 + GB) * M])
            kb_flat = kb_sb[:]

            # ---- dot + norm, 4 chunks per op to amortize instruction cost ----
            CB = 2
            dot4 = smp.tile([128, GB, NT], f32, tag="dot4")
            norm2 = smp.tile([128, GB, NT], f32, tag="norm2")
            kb_wide = kb_flat.rearrange("p (o f) -> p o f", o=1).broadcast_to(
                (128, CB, FD))
            for tb in range(NT // CB):
                t0 = tb * CB
                mc4 = mem4[:, t0 * FD:(t0 + CB) * FD]           # [128, CB*FD]
                with nc.named_scope("dotmul"):
                    prod = prodp.tile([128, CB * FD], f32, tag="prod")
                    nc.gpsimd.tensor_mul(
                        prod[:].rearrange("p (c f) -> p c f", c=CB),
                        mc4.rearrange("p (c f) -> p c f", c=CB), kb_wide)
                with nc.named_scope("dotred"):
                    nc.vector.tensor_reduce(
                        dot4[:, :, t0:t0 + CB].rearrange("p b c -> p c b"),
                        prod[:].rearrange("p (c b m) -> p c b m", c=CB, b=GB),
                        axis=AX.X, op=ALU.add,
                    )
                with nc.named_scope("norm"):
                    sq = sqp.tile([128, CB * FD], f32, tag="sq")
                    nc.scalar.activation(sq[:], mc4, ACT.Square)
                    nc.vector.tensor_reduce(
                        norm2[:, :, t0:t0 + CB].rearrange("p b c -> p c b"),
                        sq[:].rearrange("p (c b m) -> p c b m", c=CB, b=GB),
                        axis=AX.X, op=ALU.add,
                    )

            # ---- w pipeline on [128, GB, NT] tiles ----
            with nc.named_scope("wpipe"):
                lgn = smp.tile([128, GB, NT], f32, tag="t1")
                nc.scalar.activation(lgn[:], norm2[:], ACT.Ln)
                nrm = smp.tile([128, GB, NT], f32, tag="t2")
                nc.scalar.activation(nrm[:], lgn[:], ACT.Exp, scale=0.5)
                den = smp.tile([128, GB, NT], f32, tag="t3")
                nc.vector.tensor_mul(den[:], nrm[:], sc(SC_KNORM))
                rden = smp.tile([128, GB, NT], f32, tag="t5")
                nc.vector.reciprocal(rden[:], den[:])
                cos = smp.tile([128, GB, NT], f32, tag="t6")
                nc.vector.tensor_mul(cos[:], dot4[:], rden[:])
                lgt = smp.tile([128, GB, NT], f32, tag="t7")
                nc.vector.tensor_mul(lgt[:], cos[:], sc(SC_BETA))
                ex = smp.tile([128, GB, NT], f32, tag="t8")
                nc.scalar.activation(ex[:], lgt[:], ACT.Exp)
                exs = smp.tile([128, GB], f32, tag="red")
                nc.vector.tensor_reduce(exs[:], ex[:], axis=AX.X, op=ALU.add)
                ssum = wpsp.tile([1, GB], f32, tag="wps")
                nc.tensor.matmul(ssum[:], onescol_sb[:], exs[:])
                rs = smp.tile([1, GB], f32, tag="rs")
                nc.vector.reciprocal(rs[:], ssum[:])
                rsb = wpsp.tile([128, GB], f32, tag="wps")
                nc.tensor.matmul(rsb[:], ones_row, rs[:])
                wc = smp.tile([128, GB, NT], f32, tag="t9")
                nc.vector.tensor_mul(wc[:], ex[:], rsb[:].broadcast_to((128, GB, NT)))
                # interpolate
                pw4 = pwT_sb[:, b0 * NT:(b0 + GB) * NT].rearrange(
                    "p (b t) -> p b t", b=GB)
                i1 = smp.tile([128, GB, NT], f32, tag="t10")
                nc.vector.tensor_mul(i1[:], wc[:], sc(SC_G))
                i2 = smp.tile([128, GB, NT], f32, tag="t11")
                nc.vector.tensor_mul(i2[:], pw4, sc(SC_OMG))
                wg = smp.tile([128, GB, NT], f32, tag="t12")
                nc.vector.tensor_add(wg[:], i1[:], i2[:])
                wg_flat = wg[:].rearrange("p b t -> p (b t)")
                # circular shift via PE partition-shift + wrap terms as
                # accumulated selector matmuls (A[n]=wg[n-1], B[n]=wg[n+1])
                aps = wpsp.tile([128, GB * NT], f32, tag="wps")
                nc.tensor.matmul(aps[:], su_sb[:], wg_flat, start=True, stop=False)
                va = aps[:].rearrange("p (b t) -> p b t", b=GB)
                bps = wpsp.tile([128, GB * NT], f32, tag="wps")
                nc.tensor.matmul(bps[:], sd_sb[:], wg_flat, start=True, stop=False)
                vb = bps[:].rearrange("p (b t) -> p b t", b=GB)
                wgv = wg[:]
                for b in range(GB):
                    last = (b == GB - 1)
                    nc.tensor.matmul(va[:, b, 1:NT], w127_sb[:],
                                     wgv[:, b, 0:NT - 1], start=False, stop=False)
                    nc.tensor.matmul(va[:, b, 0:1], w127_sb[:],
                                     wgv[:, b, NT - 1:NT], start=False, stop=last)
                    nc.tensor.matmul(vb[:, b, 0:NT - 1], w0_sb[:],
                                     wgv[:, b, 1:NT], start=False, stop=False)
                    nc.tensor.matmul(vb[:, b, NT - 1:NT], w0_sb[:],
                                     wgv[:, b, 0:1], start=False, stop=last)
                c1 = smp.tile([128, GB, NT], f32, tag="t13")
                nc.vector.tensor_mul(c1[:], va, sc(SC_S0))
                c2 = smp.tile([128, GB, NT], f32, tag="t14")
                nc.vector.tensor_mul(c2[:], vb, sc(SC_S2))
                c3 = smp.tile([128, GB, NT], f32, tag="t15")
                nc.vector.tensor_mul(c3[:], wg[:], sc(SC_S1))
                wt1 = smp.tile([128, GB, NT], f32, tag="t16")
                nc.vector.tensor_add(wt1[:], c1[:], c2[:])
                wt2 = smp.tile([128, GB, NT], f32, tag="t17")
                nc.vector.tensor_add(wt2[:], wt1[:], c3[:])
                # sharpen: wt**y = exp(y*ln(wt))
                lw = smp.tile([128, GB, NT], f32, tag="t18")
                nc.scalar.activation(lw[:], wt2[:], ACT.Ln)
                yl = smp.tile([128, GB, NT], f32, tag="t19")
                nc.vector.tensor_mul(yl[:], lw[:], sc(SC_Y))
                pw = smp.tile([128, GB, NT], f32, tag="t20")
                nc.scalar.activation(pw[:], yl[:], ACT.Exp)
                ps2 = smp.tile([128, GB], f32, tag="red2")
                nc.vector.tensor_reduce(ps2[:], pw[:], axis=AX.X, op=ALU.add)
                ssum2 = wpsp.tile([1, GB], f32, tag="wps")
                nc.tensor.matmul(ssum2[:], onescol_sb[:], ps2[:])
                rs2 = smp.tile([1, GB], f32, tag="rs2")
                nc.vector.reciprocal(rs2[:], ssum2[:])
                rsb2 = wpsp.tile([128, GB], f32, tag="wps")
                nc.tensor.matmul(rsb2[:], ones_row, rs2[:])
                w4 = smp.tile([128, GB, NT], f32, tag="w4")
                nc.vector.tensor_mul(w4[:], pw[:], rsb2[:].broadcast_to((128, GB, NT)))
                w4_flat = w4[:].rearrange("p b t -> p (b t)")
                nc.sync.dma_start(wT_d[:, b0 * NT:(b0 + GB) * NT], w4_flat)

            # ---- read + erase/add ----
            ed_f = edp.tile([GB, FD], f32, tag="nege_f")
            nc.sync.dma_start(ed_f[:], nege_d[g])
            ad_f = edp.tile([GB, FD], f32, tag="adia_f")
            nc.sync.dma_start(ad_f[:], adia_d[g])
            ed_t = edp.tile([GB, FD], f32r, tag="nege_r")
            nc.scalar.copy(ed_t[:], ed_f[:])
            ad_t = edp.tile([GB, FD], f32r, tag="adia_r")
            nc.scalar.copy(ad_t[:], ad_f[:])
            ed = ed_t[:]
            ad = ad_t[:]
            rdps = rdpool.tile([GB, FD], f32, tag="rd")
            newm_v = newm_d[b0:b0 + GB].rearrange("b (t p) m -> p t b m", p=128)
            for tb in range(NT // CB):
                new4 = newp.tile([128, CB * FD], f32, tag="new4")
                for tc in range(CB):
                    t = tb * CB + tc
                    mc_flat = vm[:, t].rearrange("p b m -> p (b m)")
                    # transposed w rows for this chunk: [GB,128] at partition 0
                    tps = wpsp.tile([GB, 128], f32, tag="wps")
                    nc.tensor.transpose(tps[:], w4[:, :, t], id_sb[:])
                    wrows_t = smp.tile([GB, 128], f32r, tag="wrow")
                    nc.scalar.copy(wrows_t[:], tps[:])
                    wrows = wrows_t[:]
                    with nc.named_scope("write"):
                        # Single-bank erase/add: ones (start=True sets the
                        # per-element has_written bits) -> -e accumulate
                        # (bank = Q' = 1 - w x e) -> DVE in-place multiply by
                        # mem (bits persist) -> w x a accumulate -> copy out.
                        nps = ppool.tile([128, FD], f32, tag="p")
                        nc.tensor.matmul(
                            nps[:], ones_r[0:1, 0:128], ones_r[0:1, :],
                            start=True, stop=False)
                        nc.tensor.matmul(
                            nps[:], wrows, ed,
                            start=False, stop=True)
                    with nc.named_scope("read"):
                        nc.tensor.matmul(
                            rdps[:], w4[:, :, t], mc_flat,
                            start=(t == 0), stop=(t == NT - 1))
                    with nc.named_scope("erase"):
                        nc.vector.tensor_mul(nps[:], mc_flat, nps[:])
                        nc.tensor.matmul(
                            nps[:], wrows, ad,
                            start=False, stop=True, skip_group_check=True)
                        nc.scalar.copy(new4[:, tc * FD:(tc + 1) * FD], nps[:])
                with nc.named_scope("store"):
                    nv = new4[:].rearrange("p (c b m) -> p c b m", c=CB, b=GB)
                    for tc in range(CB):
                        nc.sync.dma_start(newm_v[:, tb * CB + tc], nv[:, tc])
            # read extraction: PSUM -> SBUF, then per-batch diagonal DMAs
            rdsb = smp.tile([GB, FD], f32, tag="rdout")
            nc.scalar.copy(rdsb[:], rdps[:])
            for b in range(GB):
                nc.sync.dma_start(
                    read_d[b0 + b:b0 + b + 1, :],
                    rdsb[b:b + 1, b * M:(b + 1) * M])

    nc.compile()
    return nc


def host_prep(inputs, bc=BC, ncores=NCORES):
    """Build per-core in_maps from full inputs."""
    memory = np.ascontiguousarray(inputs["memory"], dtype=np.float32)
    prev_w = np.asarray(inputs["prev_w"], dtype=np.float32)
    k = np.asarray(inputs["k"], dtype=np.float32)
    beta = np.asarray(inputs["beta"], dtype=np.float32)
    g = np.asarray(inputs["g"], dtype=np.float32)
    s = np.asarray(inputs["s"], dtype=np.float32)
    y = np.asarray(inputs["y"], dtype=np.float32)
    e = np.asarray(inputs["e"], dtype=np.float32)
    a = np.asarray(inputs["a"], dtype=np.float32)
    ng = bc // GB

    w127 = np.zeros((128, 128), dtype=np.float32)
    w127[127, 0] = 1.0                       # select row 127 -> out row 0
    w0 = np.zeros((128, 128), dtype=np.float32)
    w0[0, 127] = 1.0                         # select row 0 -> out row 127
    consts = dict(
        su=np.eye(128, k=1, dtype=np.float32),
        sd=np.eye(128, k=-1, dtype=np.float32),
        w127=w127,
        w0=w0,
        ident=np.eye(128, dtype=np.float32),
        ones=np.ones((1, FD), dtype=np.float32),
        onescol=np.ones((128, 1), dtype=np.float32),
    )
    in_maps = []
    for c in range(ncores):
        sl = slice(c * bc, (c + 1) * bc)
        pwT = prev_w[sl].reshape(bc, NT, 128).transpose(2, 0, 1).reshape(128, bc * NT)
        kbc = np.broadcast_to(k[sl].reshape(1, bc * M), (128, bc * M))
        knorm = np.linalg.norm(k[sl], axis=1)
        sc = np.stack(
            [knorm, beta[sl, 0], g[sl, 0], 1.0 - g[sl, 0],
             s[sl, 0], s[sl, 1], s[sl, 2], y[sl, 0]], axis=1)  # [bc, 8]
        scal = np.broadcast_to(sc.reshape(1, bc * NSC), (128, bc * NSC))
        nege = np.zeros((ng, GB, FD), dtype=np.float32)
        adia = np.zeros((ng, GB, FD), dtype=np.float32)
        for gg in range(ng):
            for b in range(GB):
                nege[gg, b, b * M:(b + 1) * M] = -e[sl][gg * GB + b]
                adia[gg, b, b * M:(b + 1) * M] = a[sl][gg * GB + b]
        in_maps.append(dict(
            memory=memory[sl],
            prevwT=np.ascontiguousarray(pwT),
            kbc=np.ascontiguousarray(kbc),
            scal=np.ascontiguousarray(scal, dtype=np.float32),
            negediag=nege,
            adiag=adia,
            **consts,
        ))
    return in_maps


def gather(results, bc=BC):
    """Assemble full outputs from per-core result dicts."""
    ws, reads, newms = [], [], []
    for r in results:
        wT = r["w_T"]
        ws.append(wT.reshape(128, bc, NT).transpose(1, 2, 0).reshape(bc, N))
        reads.append(r["readout"])
        newms.append(r["newmem"])
    return (
        np.concatenate(ws, axis=0),
        np.concatenate(reads, axis=0),
        np.concatenate(newms, axis=0),
    )


_CACHED_NC = None


def run(inputs, trace=False, tmpdir=None):
    global _CACHED_NC
    if _CACHED_NC is None:
        _CACHED_NC = build_program()
    nc = _CACHED_NC
    in_maps = host_prep(inputs)
    res = run_bass_kernel_spmd(
        nc, in_maps, core_ids=list(range(NCORES)), trace=trace, tmpdir=tmpdir)
    return gather(res.results), res


def kernel(**inputs):
    (w, read, new_memory), _ = run(inputs, trace=bool(os.environ.get("BASS_KERNEL_TRACE")))
    return w, read, new_memory
